# revision 5
# baseline (speedup 1.0000x reference)
"""Trainium2 Bass kernel for GQA multi-head attention (B=2, S=2048, HID=2048,
NH=32, NKV=8, HD=64), tensor-parallel over kv heads across 8 NeuronCores.

Each core c computes q-heads [4c, 4c+4) with kv-head c against the full input,
produces a partial output y_c = O_c @ Wo_c.T; the host sums the 8 partials.
"""

import sys

for _p in ("/opt/trn_rl_repo", "/root/.axon_site/_ro/trn_rl_repo"):
    if _p not in sys.path:
        sys.path.insert(0, _p)

import numpy as np

B, S, HID = 2, 2048, 2048
NH, NKV, HD = 32, 8, 64
SCALE = HD ** -0.5
NCORES = 8
NHC = NH // NCORES          # q heads per core (4)
BS = B * S                  # 4096
KT = HID // 128             # 16 contraction tiles for projections
XCHUNK = 256                # x^T chunk width (seq) per load
IT = 512                    # attention i-tile width
JT = 128                    # attention j-tile width
NEG = -1e9

_programs = {}


def _build(mode):
    """Build + compile the per-core Bass program. mode: 'causal' | 'general'."""
    import contextlib
    import concourse.bacc as bacc
    import concourse.tile as tile
    import concourse.mybir as mybir

    f32 = mybir.dt.float32
    f32r = mybir.dt.float32r
    EXP = mybir.ActivationFunctionType.Exp

    nc = bacc.Bacc("TRN2", target_bir_lowering=False, debug=False)

    xT = nc.dram_tensor("xT", [HID, BS], f32, kind="ExternalInput").ap()
    wqT = nc.dram_tensor("wqT", [HID, NHC * HD], f32, kind="ExternalInput").ap()
    wkvT = nc.dram_tensor("wkvT", [HID, 2 * HD], f32, kind="ExternalInput").ap()
    woT = nc.dram_tensor("woT", [NHC * HD, HID], f32, kind="ExternalInput").ap()
    cosQ = nc.dram_tensor("cosQ", [128, BS], f32, kind="ExternalInput").ap()
    sinQ = nc.dram_tensor("sinQ", [128, BS], f32, kind="ExternalInput").ap()
    identhi = nc.dram_tensor("identhi", [128, 64], f32, kind="ExternalInput").ap()
    onesd = nc.dram_tensor("onesd", [128, 64], f32, kind="ExternalInput").ap()
    zerod = nc.dram_tensor("zerod", [128, 128], f32, kind="ExternalInput").ap()
    if mode == "causal":
        maskadd = nc.dram_tensor("maskadd", [JT, JT], f32, kind="ExternalInput").ap()
    else:
        maskT = nc.dram_tensor("maskT", [S, BS], f32, kind="ExternalInput").ap()
        maskTr = maskT.rearrange("(J p) i -> p J i", p=128)
    y = nc.dram_tensor("y", [BS, HID], f32, kind="ExternalOutput").ap()

    xTr = xT.rearrange("(kt p) m -> p kt m", p=128)      # [128, KT, BS]
    wqTr = wqT.rearrange("(kt p) n -> p kt n", p=128)    # [128, KT, 256]
    wkvTr = wkvT.rearrange("(kt p) n -> p kt n", p=128)  # [128, KT, 128]
    woTr = woT.rearrange("(kt p) n -> p kt n", p=128)    # [128, 2, HID]

    NMI = S // XCHUNK        # x chunks per batch (8)
    NII = S // IT            # i tiles per batch (4)
    NJ = S // JT             # j tiles per batch (16)

    with tile.TileContext(nc) as tc:
        with contextlib.ExitStack() as ctx:
            singles = ctx.enter_context(tc.tile_pool(name="singles", bufs=1))
            xpool = ctx.enter_context(tc.tile_pool(name="xpool", bufs=2))
            cospool = ctx.enter_context(tc.tile_pool(name="cospool", bufs=1))
            qkv = ctx.enter_context(tc.tile_pool(name="qkv", bufs=1))
            ropetmp = ctx.enter_context(tc.tile_pool(name="ropetmp", bufs=2))
            ppool = ctx.enter_context(tc.tile_pool(name="ppool", bufs=3))
            nrm = ctx.enter_context(tc.tile_pool(name="nrm", bufs=2))
            ypool = ctx.enter_context(tc.tile_pool(name="ypool", bufs=2))
            mm_ps = ctx.enter_context(tc.tile_pool(name="mm_ps", bufs=3, space="PSUM"))
            o_ps = ctx.enter_context(tc.tile_pool(name="o_ps", bufs=2, space="PSUM"))
            v_ps = ctx.enter_context(tc.tile_pool(name="v_ps", bufs=1, space="PSUM"))
            b_ps = ctx.enter_context(tc.tile_pool(name="b_ps", bufs=1, space="PSUM"))
            if mode == "general":
                mpool = ctx.enter_context(tc.tile_pool(name="mpool", bufs=2))

            # ---- persistent weights / constants ----
            wq_s = singles.tile([128, KT, NHC * HD], f32r)
            nc.sync.dma_start(out=wq_s, in_=wqTr.bitcast(f32r))
            wkv_s = singles.tile([128, KT, 2 * HD], f32r)
            nc.sync.dma_start(out=wkv_s, in_=wkvTr.bitcast(f32r))
            wo_s = singles.tile([128, 2, HID], f32r)
            nc.sync.dma_start(out=wo_s, in_=woTr.bitcast(f32r))
            if mode == "causal":
                mask_s = singles.tile([JT, JT], f32)
                nc.sync.dma_start(out=mask_s, in_=maskadd)
            ident_hi = singles.tile([128, 64], f32)
            nc.sync.dma_start(out=ident_hi, in_=identhi)
            ones_t = singles.tile([128, 64], f32r)
            nc.sync.dma_start(out=ones_t, in_=onesd.bitcast(f32r))
            zero_r = singles.tile([128, 128], f32r)
            nc.sync.dma_start(out=zero_r, in_=zerod.bitcast(f32r))

            for b in range(B):
                cb = b * S
                # per-batch rope tables ([c; c] rows, sin sign-folded)
                cq_s = cospool.tile([128, S], f32, tag="cq")
                sq_s = cospool.tile([128, S], f32, tag="sq")
                nc.sync.dma_start(out=cq_s, in_=cosQ[:, cb:cb + S])
                nc.sync.dma_start(out=sq_s, in_=sinQ[:, cb:cb + S])

                qT_s = qkv.tile([64, NHC, S], f32r, tag="qT")
                kT_s = qkv.tile([64, S], f32r, tag="kT")
                v_s = qkv.tile([128, NJ, HD + 1], f32r, tag="v")
                nc.sync.dma_start(
                    out=v_s[:, :, HD:HD + 1],
                    in_=onesd[:, 0:NJ].rearrange("p (a b) -> p a b", b=1).bitcast(f32r))
                oT_s = qkv.tile([128, 2, S], f32r, tag="oT")

                # ---- phase A: projections + RoPE + V transpose ----
                for mi in range(NMI):
                    m0 = mi * XCHUNK
                    msl = slice(m0, m0 + XCHUNK)
                    xt = xpool.tile([128, KT, XCHUNK], f32r, tag="xt")
                    nc.sync.dma_start(
                        out=xt, in_=xTr[:, :, cb + m0:cb + m0 + XCHUNK].bitcast(f32r))

                    for ni in range(2):
                        p_q = mm_ps.tile([128, IT], f32, tag="mm")
                        for kt in range(KT):
                            nc.tensor.matmul(
                                p_q[:, :XCHUNK],
                                wq_s[:, kt, ni * 128:(ni + 1) * 128],
                                xt[:, kt, :],
                                start=(kt == 0), stop=(kt == KT - 1))
                        q_raw = ropetmp.tile([128, XCHUNK], f32, tag="qraw")
                        nc.scalar.copy(q_raw, p_q[:, :XCHUNK])
                        # RoPE: out = x*cos + shift_half(x)*sin_signed
                        t_c = ropetmp.tile([128, XCHUNK], f32, tag="tc")
                        t_s = ropetmp.tile([128, XCHUNK], f32, tag="ts")
                        nc.vector.tensor_mul(t_c, q_raw, cq_s[:, msl])
                        for r0 in (0, 64):
                            nc.vector.tensor_mul(
                                t_s[r0:r0 + 32], q_raw[r0 + 32:r0 + 64],
                                sq_s[r0 + 32:r0 + 64, msl])
                            nc.vector.tensor_mul(
                                t_s[r0 + 32:r0 + 64], q_raw[r0:r0 + 32],
                                sq_s[r0:r0 + 32, msl])
                        nc.vector.tensor_add(
                            qT_s[:, 2 * ni, msl], t_c[0:64], t_s[0:64])
                        nc.vector.tensor_add(
                            qT_s[:, 2 * ni + 1, msl], t_c[64:128], t_s[64:128])

                    p_kv = mm_ps.tile([128, IT], f32, tag="mm")
                    for kt in range(KT):
                        nc.tensor.matmul(
                            p_kv[:, :XCHUNK], wkv_s[:, kt, :], xt[:, kt, :],
                            start=(kt == 0), stop=(kt == KT - 1))
                    kv_raw = ropetmp.tile([128, XCHUNK], f32, tag="qraw")
                    nc.scalar.copy(kv_raw, p_kv[:, :XCHUNK])
                    t_c = ropetmp.tile([128, XCHUNK], f32, tag="tc")
                    t_s = ropetmp.tile([128, XCHUNK], f32, tag="ts")
                    nc.vector.tensor_mul(t_c[0:64], kv_raw[0:64], cq_s[0:64, msl])
                    nc.vector.tensor_mul(
                        t_s[0:32], kv_raw[32:64], sq_s[32:64, msl])
                    nc.vector.tensor_mul(
                        t_s[32:64], kv_raw[0:32], sq_s[0:32, msl])
                    nc.vector.tensor_add(kT_s[:, msl], t_c[0:64], t_s[0:64])
                    # V transpose to seq-major with ones column for rowsums
                    for jj2 in range(XCHUNK // JT):
                        jt = (m0 // JT) + jj2
                        p_v = v_ps.tile([128, 64], f32, tag="vt")
                        nc.tensor.transpose(
                            p_v, kv_raw[64:128, jj2 * JT:(jj2 + 1) * JT],
                            ident_hi[64:128, :])
                        nc.vector.tensor_copy(v_s[:, jt, 0:HD], p_v)

                # ---- phase B: attention ----
                for ii in range(NII):
                    i0 = ii * IT
                    jmax = 4 * ii + 3 if mode == "causal" else NJ - 1
                    if mode == "general":
                        mk_s = mpool.tile([128, NJ, IT], f32, tag="mk")
                        nc.sync.dma_start(
                            out=mk_s, in_=maskTr[:, :, cb + i0:cb + i0 + IT])
                    for h in range(NHC):
                        p_o = o_ps.tile([HD + 1, IT], f32, tag="po")
                        for J in range(jmax + 1):
                            ksl = kT_s[:, J * JT:(J + 1) * JT]
                            pt = ppool.tile([128, IT], f32r, tag="pt")
                            p_s = mm_ps.tile([128, IT], f32, tag="mm")
                            if mode == "general" or J < 4 * ii:
                                c0, cm = 0, 0
                            else:
                                r = J - 4 * ii
                                c0 = r * JT          # first valid column
                                cm = min(c0, IT - 2 * JT)  # matmul range >=256 wide
                            nc.tensor.matmul(
                                p_s[:, cm:IT], ksl,
                                qT_s[:, h, i0 + cm:i0 + IT],
                                start=True, stop=True)
                            if mode == "general":
                                nc.vector.tensor_add(p_s, p_s, mk_s[:, J, :])
                                nc.scalar.activation(pt, p_s, EXP)
                            elif J < 4 * ii:
                                nc.scalar.activation(pt, p_s, EXP)
                            else:
                                nc.vector.tensor_add(
                                    p_s[:, c0:c0 + JT], p_s[:, c0:c0 + JT], mask_s)
                                if c0 > cm:
                                    nc.vector.tensor_copy(
                                        pt[:, cm:c0], zero_r[:, 0:c0 - cm])
                                nc.scalar.activation(
                                    pt[:, c0:IT], p_s[:, c0:IT], EXP)
                            nc.tensor.matmul(
                                p_o[:, cm:IT], v_s[:, J, :], pt[:, cm:IT],
                                start=(J == 0), stop=(J == jmax),
                                skip_group_check=True)
                        # normalize: O^T = Ō^T * (1/rowsum), rowsum in p_o row 64
                        rc_t = nrm.tile([65, IT], f32, tag="rc")
                        nc.vector.reciprocal(rc_t[64:65, :], p_o[HD:HD + 1, :])
                        rcr_t = nrm.tile([65, IT], f32r, tag="rcr")
                        nc.vector.tensor_copy(rcr_t[64:65, :], rc_t[64:65, :])
                        p_b = b_ps.tile([64, IT], f32, tag="pb")
                        nc.tensor.matmul(
                            p_b, ones_t[64:65, :], rcr_t[64:65, :],
                            start=True, stop=True)
                        rb_s = nrm.tile([64, IT], f32, tag="rb")
                        nc.scalar.copy(rb_s, p_b)
                        ntile, hr = h // 2, (h % 2) * 64
                        if hr == 0:
                            nc.vector.tensor_mul(
                                oT_s[0:64, ntile, i0:i0 + IT], p_o[0:HD, :], rb_s)
                        else:
                            otmp = nrm.tile([64, IT], f32r, tag="otmp")
                            nc.vector.tensor_mul(otmp, p_o[0:HD, :], rb_s)
                            nc.vector.tensor_copy(
                                oT_s[64:128, ntile, i0:i0 + IT], otmp)

                # ---- phase C: output projection ----
                for mi2 in range(S // 128):
                    m0 = mi2 * 128
                    for nh2 in range(2):
                        ys = ypool.tile([128, HID // 2], f32, tag="ys")
                        for ni2 in range(2):
                            n0 = nh2 * (HID // 2) + ni2 * IT
                            p_y = mm_ps.tile([128, IT], f32, tag="mm")
                            for kt2 in range(2):
                                nc.tensor.matmul(
                                    p_y, oT_s[:, kt2, m0:m0 + 128],
                                    wo_s[:, kt2, n0:n0 + IT],
                                    start=(kt2 == 0), stop=(kt2 == 1))
                            if (mi2 + ni2) % 2 == 0:
                                nc.vector.tensor_copy(
                                    ys[:, ni2 * IT:(ni2 + 1) * IT], p_y)
                            else:
                                nc.scalar.copy(ys[:, ni2 * IT:(ni2 + 1) * IT], p_y)
                        nc.sync.dma_start(
                            out=y[cb + m0:cb + m0 + 128,
                                  nh2 * (HID // 2):(nh2 + 1) * (HID // 2)],
                            in_=ys)

    nc.compile()
    return nc


def _get_program(mode):
    if mode not in _programs:
        _programs[mode] = _build(mode)
    return _programs[mode]


def _host_prep(hidden_states, cos, sin, Wq, Wk, Wv, Wo):
    """Shared (non-per-core) host-side layout prep."""
    f32 = np.float32
    X = np.ascontiguousarray(hidden_states.reshape(BS, HID).T).astype(f32, copy=False)
    # cos/sin tables, transposed, batch-major columns: [64, BS]
    cosT = np.concatenate([cos[b].T for b in range(B)], axis=1).astype(f32)
    sinT = np.concatenate([sin[b].T for b in range(B)], axis=1).astype(f32)
    sinS = np.concatenate([sinT[0:HD // 2], -sinT[0:HD // 2]], axis=0)
    cosQ = np.ascontiguousarray(np.concatenate([cosT, cosT], axis=0))
    sinQ = np.ascontiguousarray(np.concatenate([sinS, sinS], axis=0))
    per_core = []
    for c in range(NCORES):
        wq_c = Wq[c * NHC * HD:(c + 1) * NHC * HD, :] * SCALE
        wqT = np.ascontiguousarray(wq_c.T.astype(f32))
        wk_c = Wk[c * HD:(c + 1) * HD, :]
        wv_c = Wv[c * HD:(c + 1) * HD, :]
        wkvT = np.ascontiguousarray(np.concatenate([wk_c, wv_c], axis=0).T.astype(f32))
        woT = np.ascontiguousarray(Wo[:, c * NHC * HD:(c + 1) * NHC * HD].T.astype(f32))
        per_core.append((wqT, wkvT, woT))
    identhi = np.zeros((128, 64), dtype=f32)
    identhi[64:128, :] = np.eye(64, dtype=f32)
    onesd = np.ones((128, 64), dtype=f32)
    zerod = np.zeros((128, 128), dtype=f32)
    return X, cosQ, sinQ, identhi, onesd, zerod, per_core


def _is_causal(attention_mask):
    am = np.asarray(attention_mask)
    if am.shape != (B, 1, S, S):
        return False
    tri = np.where(np.tril(np.ones((S, S), dtype=bool)),
                   np.float32(0.0), np.float32(NEG))
    return bool(np.array_equal(am[0, 0], tri) and np.array_equal(am[1, 0], tri))


def _make_in_maps(inputs_f32, causal):
    hidden_states, cos, sin, attention_mask, Wq, Wk, Wv, Wo = inputs_f32
    X, cosQ, sinQ, identhi, onesd, zerod, per_core = _host_prep(
        hidden_states, cos, sin, Wq, Wk, Wv, Wo)
    jj = np.arange(JT, dtype=np.float32)
    madd = np.where(jj[None, :] >= jj[:, None], 0.0, NEG).astype(np.float32)
    in_maps = []
    for c in range(NCORES):
        wqT, wkvT, woT = per_core[c]
        m = {"xT": X, "wqT": wqT, "wkvT": wkvT, "woT": woT,
             "cosQ": cosQ, "sinQ": sinQ, "identhi": identhi,
             "onesd": onesd, "zerod": zerod}
        if causal:
            m["maskadd"] = madd
        else:
            mT = np.concatenate(
                [attention_mask[b, 0].T for b in range(B)], axis=1)
            m["maskT"] = np.ascontiguousarray(mT)
        in_maps.append(m)
    return in_maps


def kernel(hidden_states, cos, sin, attention_mask, Wq, Wk, Wv, Wo):
    from concourse.bass_utils import run_bass_kernel_spmd

    inputs_f32 = tuple(
        np.asarray(a, dtype=np.float32)
        for a in (hidden_states, cos, sin, attention_mask, Wq, Wk, Wv, Wo))

    causal = _is_causal(inputs_f32[3])
    nc = _get_program("causal" if causal else "general")
    in_maps = _make_in_maps(inputs_f32, causal)

    res = run_bass_kernel_spmd(nc, in_maps, core_ids=list(range(NCORES)))
    acc = np.zeros((BS, HID), dtype=np.float64)
    for c in range(NCORES):
        acc += res.results[c]["y"].astype(np.float64)
    return acc.astype(np.float32).reshape(B, S, HID)
